# revision 1
# baseline (speedup 1.0000x reference)
"""Causal multi-head self-attention (B=1, S=4096, D=1024, H=16) on 8 NeuronCores.

Sharding: tensor-parallel over heads — each core owns 2 heads (Wq/Wk/Wv column
slices, Wo row slice), computes a partial output projection, and the host sums
the 8 partials.

Device-side design (per core):
  - Host pre-transposes x -> xT [D, S] and pre-permutes/stacks weights so no
    on-device layout shuffles are needed for q/k.
  - qT/kT computed in [channel, seq] layout (rows = [h0_x1|h1_x1|h0_x2|h1_x2],
    channels de-interleaved even/odd so RoPE is two contiguous 64-row halves).
  - v computed in [ch, seq] then PE-transposed to [seq, ch] blocks (PV matmul
    needs V as the stationary operand in natural layout).
  - Attention in scores^T layout: scores^T[sk,sq] = K_blk @ Q_blk^T, softmax
    without max-subtraction (scores are O(±8) for this distribution so exp is
    safe in fp32), row-sums obtained by appending a ones-column to V, causal
    masking via one static 128x128 triangle tile + gpsimd memsets on fully
    masked column ranges.
  - All matmuls run as float32r (full PE rate, fp32 storage).
"""

import os
import sys

import numpy as np

for _p in ("/opt/trn_rl_repo", "/root/.axon_site/_ro/trn_rl_repo"):
    if os.path.isdir(_p) and _p not in sys.path:
        sys.path.insert(0, _p)

import concourse.bass as bass
import concourse.mybir as mybir
import concourse.tile as tile
from concourse import bacc
from concourse.bass_utils import run_bass_kernel_spmd
from concourse.masks import make_identity


def _install_ntff_shim():
    """The agent image's antenv lacks axon_hooks; provide it so
    run_bass_kernel_spmd(trace=True) can capture NTFF profiles."""
    try:
        from antenv import axon_hooks  # noqa: F401
        return
    except ImportError:
        pass
    try:
        import types
        import antenv
        from trn_agent_boot.trn_boot import _ntff_profile_via_ctypes
        so = "/opt/axon/libaxon_pjrt.so"
        if not os.path.exists(so):
            return
        hook = _ntff_profile_via_ctypes(so)
        m = types.ModuleType("antenv.axon_hooks")
        m._hook = hook
        m.set_axon_ntff_profile_hook = lambda h: setattr(m, "_hook", h)
        m.get_axon_ntff_profile_hook = lambda: m._hook
        sys.modules["antenv.axon_hooks"] = m
        antenv.axon_hooks = m
    except Exception:
        pass


_install_ntff_shim()

F32 = mybir.dt.float32
F32R = mybir.dt.float32r

S = 4096
D = 1024
H = 16
DK = 64
N_CORES = 8
SQ = 512          # q-block width (PSUM bank limit for fp32 N)
SK = 128          # k-block width (partition dim of scores^T)
GK = 2            # k-blocks fused per exp group (2 PSUM banks)
NB512 = S // SQ   # 8
NB128 = S // SK   # 32


def _emit(tc, xT, wqkT, wvT, woT, cosT, sinT, tri, y):
    nc = tc.nc
    ctx_pools = []
    PHASES = int(os.environ.get("KERN_PHASES", "5"))

    # ---------------- persistent SBUF ----------------
    const = tc.tile_pool(name="const", bufs=1)
    big = tc.tile_pool(name="big", bufs=1)
    cp = const.__enter__()
    bp = big.__enter__()
    ctx_pools += [const, big]

    wqk_sb = cp.tile([128, 8, 256], F32R, tag="wqk")      # [part, kchunk, 256]
    wv_sb = cp.tile([128, 8, 128], F32R, tag="wv")
    wo_sb = cp.tile([128, 1024], F32R, tag="wo")
    cos_sb = cp.tile([128, S], F32, tag="cos")
    sin_sb = cp.tile([128, S], F32, tag="sin")
    tri_sb = cp.tile([128, 512], F32, tag="tri")
    ident = cp.tile([128, 128], F32, tag="ident")
    ones_sb = cp.tile([65, 64], F32R, tag="ones")

    nc.sync.dma_start(wqk_sb[:], wqkT.ap().rearrange("(c p) n -> p c n", p=128))
    nc.sync.dma_start(wv_sb[:], wvT.ap().rearrange("(c p) n -> p c n", p=128))
    nc.sync.dma_start(wo_sb[:], woT.ap())
    nc.sync.dma_start(cos_sb[:], cosT.ap())
    nc.sync.dma_start(sin_sb[:], sinT.ap())
    nc.sync.dma_start(tri_sb[:], tri.ap())
    make_identity(nc, ident[:])
    ones_f32 = cp.tile([65, 64], F32, tag="ones_f32")
    nc.vector.memset(ones_f32[:], 1.0)
    nc.vector.tensor_copy(ones_sb[64:65, :], ones_f32[64:65, :])

    qT = bp.tile([128, S], F32R, tag="qT")
    kT = bp.tile([128, S], F32R, tag="kT")
    vt_sb = bp.tile([128, S], F32, tag="vt_out")         # vT, later reused for outT
    v_h = [bp.tile([128, NB128 * 65], F32R, tag=f"v{h}", name=f"v{h}")
           for h in range(2)]

    # ---------------- phase B: projections + interleaved RoPE ----------------
    # RoPE row layout: [h0_x1(32) | h0_x2(32) | h1_x1(32) | h1_x2(32)]; the
    # +- sign is folded into the host-built sin table (x1 rows carry -sin):
    #   ta = t * cos2;  tb = sw * sin2_signed;  t = ta + tb
    # RoPE for the first half of the sequence is emitted as soon as the
    # projections for those columns are done, so attention q-blocks 0..3
    # can start while the second half is still projecting.
    mul = mybir.AluOpType.mult
    HW = 2048  # RoPE free-dim chunk

    rope_pool = tc.tile_pool(name="rope", bufs=2)
    rp = rope_pool.__enter__()
    ctx_pools.append(rope_pool)
    sws = {0: rp.tile([128, HW], F32R, tag="swq", name="swq", bufs=2),
           1: rp.tile([128, HW], F32R, tag="swk", name="swk", bufs=2)}

    def emit_rope(chunk):
        sl = slice(HW * chunk, HW * (chunk + 1))
        for ti, t in enumerate((qT, kT)):
            sw = rp.tile([128, HW], F32R, tag=("swq", "swk")[ti],
                         name=f"sw{ti}_{chunk}", bufs=2)
            for blk in range(4):
                dst = slice(32 * blk, 32 * blk + 32)
                srcs = slice(32 * (blk ^ 1), 32 * (blk ^ 1) + 32)
                nc.sync.dma_start(sw[dst, :], t[srcs, sl])
            ta = rp.tile([128, HW], F32, tag="ta")
            tb = rp.tile([128, HW], F32, tag="tb")
            nc.vector.tensor_tensor(ta[:], t[:, sl], cos_sb[:, sl], op=mul)
            nc.vector.tensor_tensor(tb[:], sw[:], sin_sb[:, sl], op=mul)
            nc.vector.tensor_tensor(t[:, sl], ta[:], tb[:],
                                    op=mybir.AluOpType.add)

    with tc.tile_pool(name="xp", bufs=12) as xp, \
         tc.tile_pool(name="prps", bufs=6, space="PSUM") as prps:
        for sb in range(NB512):
            xts = []
            for kc in range(8):
                xt = xp.tile([128, SQ], F32R, tag="x")
                nc.sync.dma_start(
                    xt[:], xT.ap()[128 * kc:128 * (kc + 1), SQ * sb:SQ * (sb + 1)])
                xts.append(xt)
            q_ps = prps.tile([128, SQ], F32, tag="pr")
            k_ps = prps.tile([128, SQ], F32, tag="pr")
            v_ps = prps.tile([128, SQ], F32, tag="pr")
            for kc in range(8):
                st, sp = (kc == 0), (kc == 7)
                nc.tensor.matmul(q_ps[:], wqk_sb[:, kc, 0:128], xts[kc][:],
                                 start=st, stop=sp)
                nc.tensor.matmul(k_ps[:], wqk_sb[:, kc, 128:256], xts[kc][:],
                                 start=st, stop=sp)
                nc.tensor.matmul(v_ps[:], wv_sb[:, kc, :], xts[kc][:],
                                 start=st, stop=sp)
            sl = slice(SQ * sb, SQ * (sb + 1))
            nc.vector.tensor_copy(qT[:, sl], q_ps[:])
            nc.vector.tensor_copy(kT[:, sl], k_ps[:])
            nc.vector.tensor_copy(vt_sb[:, sl], v_ps[:])
            if sb == 3:
                emit_rope(0)
        emit_rope(1)

    if PHASES < 2:
        for p in reversed(ctx_pools):
            p.__exit__(None, None, None)
        return
    # ---------------- phase C: v^T -> v blocks (PE transpose) ----------------
    # ones column for row-sums: v_h[:, 65j+64] = 1.0
    onec = cp.tile([128, 1], F32, tag="onec")
    nc.vector.memset(onec[:], 1.0)
    for h in range(2):
        ones_col = v_h[h][:].rearrange("p (b c) -> p b c", c=65)[:, :, 64]
        nc.vector.tensor_copy(ones_col, onec[:].broadcast_to([128, 32]))
    with tc.tile_pool(name="vtp", bufs=4, space="PSUM") as vtp:
        for j in range(NB128):
            tp = vtp.tile([128, 128], F32, tag="vt")
            nc.tensor.transpose(tp[:], vt_sb[:, 128 * j:128 * (j + 1)], ident[:])
            for h in range(2):
                nc.vector.tensor_copy(v_h[h][:, 65 * j:65 * j + 64],
                                      tp[:, 64 * h:64 * h + 64])

    if PHASES < 3:
        for p in reversed(ctx_pools):
            p.__exit__(None, None, None)
        return
    if PHASES < 4:
        for p in reversed(ctx_pools):
            p.__exit__(None, None, None)
        return
    # ---------------- phase D: attention + interleaved output projection ----
    DMODE = int(os.environ.get("KERN_DMODE", "4"))
    outT = bp.tile([128, S], F32R, tag="vt_out")  # reuses vT slot
    exp = mybir.ActivationFunctionType.Exp
    with tc.tile_pool(name="scps", bufs=2, space="PSUM") as scps, \
         tc.tile_pool(name="smps", bufs=1, space="PSUM") as smps, \
         tc.tile_pool(name="yps", bufs=3, space="PSUM") as yps, \
         tc.tile_pool(name="ptp", bufs=3) as ptp, \
         tc.tile_pool(name="ysb", bufs=4) as ysb, \
         tc.tile_pool(name="recp", bufs=2) as recp:
        pending = [None]

        def flush_norm():
            if pending[0] is None:
                return
            h_, b_, out_ps_ = pending[0]
            pending[0] = None
            qsl_ = slice(SQ * b_, SQ * (b_ + 1))
            rec = recp.tile([65, SQ], F32R, tag="rec", name="rec")
            with nc.allow_low_precision(reason="fp32r reciprocal for bcast"):
                nc.vector.reciprocal(rec[64:65, :], out_ps_[64:65, :])
            bc = yps.tile([64, SQ], F32, tag="y", name="bc")
            nc.tensor.matmul(bc[:], ones_sb[64:65, :], rec[64:65, :],
                             start=True, stop=True)
            if h_ == 0:
                dst = outT[0:64, qsl_]
                nc.vector.tensor_copy(dst, out_ps_[0:64, :])
                nc.vector.tensor_tensor(dst, dst, bc[:], op=mul)
            else:
                tmp64 = recp.tile([64, SQ], F32R, tag="tmp64", name="tmp64")
                nc.vector.tensor_copy(tmp64[:], out_ps_[0:64, :])
                nc.vector.tensor_tensor(tmp64[:], tmp64[:], bc[:], op=mul)
                nc.sync.dma_start(outT[64:128, qsl_], tmp64[:])
                # outT block b_ now complete -> emit its output projection
                for m in range(4 * b_, 4 * b_ + 4):
                    msl = slice(128 * m, 128 * (m + 1))
                    for nh in range(2):
                        nsl = slice(512 * nh, 512 * (nh + 1))
                        y_ps = yps.tile([128, SQ], F32, tag="y", name="y_ps")
                        nc.tensor.matmul(y_ps[:], outT[:, msl], wo_sb[:, nsl],
                                         start=True, stop=True)
                        y_sb = ysb.tile([128, SQ], F32, tag="ysb", name="y_sb")
                        if (m + nh) % 2 == 0:
                            nc.vector.tensor_copy(y_sb[:], y_ps[:])
                        else:
                            nc.scalar.copy(y_sb[:], y_ps[:])
                        nc.sync.dma_start(y.ap()[msl, nsl], y_sb[:])

        for b in (3, 7, 6, 5, 4, 2, 1, 0):
            nk = 4 * b + 4
            qsl = slice(SQ * b, SQ * (b + 1))
            for h in range(2):
                out_ps = smps.tile([65, SQ], F32, tag="sm")
                rh = slice(64 * h, 64 * h + 64)          # head h d_k rows
                for g0 in range(0, nk, GK):
                    gw = min(GK, nk - g0)
                    sc = scps.tile([128, GK * SQ], F32, tag="sc")
                    for j2 in range(gw):
                        k = g0 + j2
                        ksl = slice(SK * k, SK * (k + 1))
                        osl = slice(SQ * j2, SQ * (j2 + 1))
                        nc.tensor.matmul(sc[:, osl], kT[rh, ksl], qT[rh, qsl],
                                         start=True, stop=True)
                    pt = ptp.tile([128, GK * SQ], F32R, tag="pt")
                    nc.scalar.activation(pt[:, 0:SQ * gw], sc[:, 0:SQ * gw], exp,
                                         scale=0.125)
                    for j2 in range(gw):
                        k = g0 + j2
                        if k >= 4 * b:               # diagonal-straddling block
                            j = k - 4 * b
                            c0 = SQ * j2
                            w = 128 * j + 128
                            dsl = slice(c0, c0 + w)
                            nc.vector.tensor_tensor(pt[:, dsl], pt[:, dsl],
                                                    tri_sb[:, 512 - w:512],
                                                    op=mul)
                    if g0 == GK:
                        flush_norm()
                    for j2 in range(gw):
                        k = g0 + j2
                        nc.tensor.matmul(out_ps[:],
                                         v_h[h][:, 65 * k:65 * k + 65],
                                         pt[:, SQ * j2:SQ * (j2 + 1)],
                                         start=(k == 0), stop=(k == nk - 1))
                pending[0] = (h, b, out_ps)
        flush_norm()

    for p in reversed(ctx_pools):
        p.__exit__(None, None, None)


_CACHED = None


def _build():
    global _CACHED
    if _CACHED is not None:
        return _CACHED
    nc = bacc.Bacc("TRN2", target_bir_lowering=False, debug=False)
    xT = nc.dram_tensor("xT", [D, S], F32R, kind="ExternalInput")
    wqkT = nc.dram_tensor("wqkT", [D, 256], F32R, kind="ExternalInput")
    wvT = nc.dram_tensor("wvT", [D, 128], F32R, kind="ExternalInput")
    woT = nc.dram_tensor("woT", [128, D], F32R, kind="ExternalInput")
    cosT = nc.dram_tensor("cosT", [128, S], F32, kind="ExternalInput")
    sinT = nc.dram_tensor("sinT", [128, S], F32, kind="ExternalInput")
    tri = nc.dram_tensor("tri", [128, 512], F32, kind="ExternalInput")
    y = nc.dram_tensor("y", [S, D], F32, kind="ExternalOutput")
    with tile.TileContext(nc) as tc:
        _emit(tc, xT, wqkT, wvT, woT, cosT, sinT, tri, y)
    nc.compile()
    _CACHED = nc
    return nc


def _host_prep(x, token_positions, Wq, Wk, Wv, Wo):
    x = np.asarray(x, dtype=np.float32).reshape(S, D)
    xT = np.ascontiguousarray(x.T)

    pos = np.asarray(token_positions).reshape(S).astype(np.float32)
    inv = (np.float32(10000.0) **
           (-np.arange(0, DK // 2, dtype=np.float32) * np.float32(2.0 / DK)))
    ang = pos[None, :] * inv[:, None]          # [32, S]
    cosF = np.cos(ang).astype(np.float32)
    sinF = np.sin(ang).astype(np.float32)
    cosT = np.ascontiguousarray(np.tile(cosF, (4, 1)))          # [128, S]
    sinT = np.ascontiguousarray(np.tile(
        np.concatenate([-sinF, sinF], axis=0), (2, 1)))          # signed

    ii = np.arange(128)[:, None]
    uu = np.arange(512)[None, :]
    tri = (uu >= ii + 384).astype(np.float32)   # strip mask B01 [128, 512]

    Wq = np.asarray(Wq, dtype=np.float32)
    Wk = np.asarray(Wk, dtype=np.float32)
    Wv = np.asarray(Wv, dtype=np.float32)
    Wo = np.asarray(Wo, dtype=np.float32)

    in_maps = []
    for c in range(N_CORES):
        idx = []
        for hl in range(2):   # per head: 32 even channels then 32 odd channels
            idx += [64 * (2 * c + hl) + 2 * j for j in range(32)]
            idx += [64 * (2 * c + hl) + 2 * j + 1 for j in range(32)]
        wq_c = Wq[idx, :]                       # [128, 1024]
        wk_c = Wk[idx, :]
        wqkT = np.ascontiguousarray(
            np.concatenate([wq_c.T, wk_c.T], axis=1))      # [1024, 256]
        wvT = np.ascontiguousarray(Wv[128 * c:128 * (c + 1), :].T)  # [1024, 128]
        woT = np.ascontiguousarray(Wo[:, 128 * c:128 * (c + 1)].T)  # [128, 1024]
        in_maps.append({
            "xT": xT, "wqkT": wqkT, "wvT": wvT, "woT": woT,
            "cosT": cosT, "sinT": sinT, "tri": tri,
        })
    return in_maps


def run(x, token_positions, Wq, Wk, Wv, Wo, trace=False):
    nc = _build()
    in_maps = _host_prep(x, token_positions, Wq, Wk, Wv, Wo)
    res = run_bass_kernel_spmd(nc, in_maps, core_ids=list(range(N_CORES)),
                               trace=trace)
    y = np.zeros((S, D), dtype=np.float32)
    for c in range(N_CORES):
        y += res.results[c]["y"]
    return y.reshape(1, S, D), res


def kernel(x, token_positions, Wq, Wk, Wv, Wo):
    y, _ = run(x, token_positions, Wq, Wk, Wv, Wo)
    return y



# revision 8
# speedup vs baseline: 1.1556x; 1.1556x over previous
"""Causal multi-head self-attention (B=1, S=4096, D=1024, H=16) on 8 NeuronCores.

Sharding: tensor-parallel over heads — each core owns 2 heads (Wq/Wk/Wv column
slices, Wo row slice), computes a partial output projection, and the host sums
the 8 partials.

v2: all matmul operands in bf16 (PSUM accum stays fp32) to dodge the fp32r
power throttle and halve DMA; reciprocal via the fast custom-DVE approx;
causal mask multiplies only the 128 partial columns of each diagonal block;
out_ps double-buffered so consecutive (b,h) attention blocks overlap; phase-B
PSUM drains moved to the (idle) scalar engine.

Device-side design (per core):
  - Host pre-transposes x -> xT [D, S] (bf16) and pre-permutes/stacks weights
    so no on-device layout shuffles are needed for q/k.
  - qT/kT computed in [channel, seq] layout (rows = [h0_x1|h0_x2|h1_x1|h1_x2],
    channels de-interleaved even/odd so RoPE is contiguous 32-row quarters).
  - v computed in [ch, seq] then PE-transposed to [seq, ch] blocks; both heads
    live in one v_all tile ([128, 130] per k-block: dk64+ones per head).
  - Attention in scores^T layout: scores^T[sk,sq] = K_blk @ Q_blk^T, softmax
    without max-subtraction (scores are O(±8)), row-sums via a ones-column in
    V, causal masking via one static 128x128 triangle on the partial region.
"""

import os
import sys

import numpy as np

for _p in ("/opt/trn_rl_repo", "/root/.axon_site/_ro/trn_rl_repo"):
    if os.path.isdir(_p) and _p not in sys.path:
        sys.path.insert(0, _p)

import ml_dtypes

import concourse.bass as bass
import concourse.mybir as mybir
import concourse.tile as tile
from concourse import bacc
from concourse.bass_utils import run_bass_kernel_spmd
from concourse.masks import make_identity


def _install_ntff_shim():
    """The agent image's antenv lacks axon_hooks; provide it so
    run_bass_kernel_spmd(trace=True) can capture NTFF profiles."""
    try:
        from antenv import axon_hooks  # noqa: F401
        return
    except ImportError:
        pass
    try:
        import types
        import antenv
        from trn_agent_boot.trn_boot import _ntff_profile_via_ctypes
        so = "/opt/axon/libaxon_pjrt.so"
        if not os.path.exists(so):
            return
        hook = _ntff_profile_via_ctypes(so)
        m = types.ModuleType("antenv.axon_hooks")
        m._hook = hook
        m.set_axon_ntff_profile_hook = lambda h: setattr(m, "_hook", h)
        m.get_axon_ntff_profile_hook = lambda: m._hook
        sys.modules["antenv.axon_hooks"] = m
        antenv.axon_hooks = m
    except Exception:
        pass


_install_ntff_shim()

F32 = mybir.dt.float32
F32R = mybir.dt.float32r
BF16 = mybir.dt.bfloat16
NPBF16 = ml_dtypes.bfloat16

S = 4096
D = 1024
H = 16
DK = 64
N_CORES = 8
SQ = 512          # q-block width (PSUM bank limit for fp32 N)
SK = 128          # k-block width (partition dim of scores^T)
GK = 2            # k-blocks fused per exp group (2 PSUM banks)
NB512 = S // SQ   # 8
NB128 = S // SK   # 32


def _emit(tc, xT, wqkT, wvT, woT, cosT, sinT, tri, y, dbg=None):
    nc = tc.nc
    ctx_pools = []

    # ---------------- persistent SBUF ----------------
    const = tc.tile_pool(name="const", bufs=1)
    big = tc.tile_pool(name="big", bufs=1)
    cp = const.__enter__()
    bp = big.__enter__()
    ctx_pools += [const, big]

    wqk_sb = cp.tile([128, 8, 256], BF16, tag="wqk")      # [part, kchunk, 256]
    wv_sb = cp.tile([128, 8, 128], BF16, tag="wv")
    wo_sb = cp.tile([128, 1024], BF16, tag="wo")
    cos_sb = cp.tile([128, S], BF16, tag="cos")
    sin_sb = cp.tile([128, S], BF16, tag="sin")
    tri_sb = cp.tile([128, 128], BF16, tag="tri")
    ident_f = cp.tile([128, 128], F32, tag="ident_f")
    ident = cp.tile([128, 128], BF16, tag="ident")
    ones_sb = cp.tile([65, 64], BF16, tag="ones")

    nc.sync.dma_start(wqk_sb[:], wqkT.ap().rearrange("(c p) n -> p c n", p=128))
    nc.sync.dma_start(wv_sb[:], wvT.ap().rearrange("(c p) n -> p c n", p=128))
    nc.sync.dma_start(wo_sb[:], woT.ap())
    nc.sync.dma_start(cos_sb[:], cosT.ap())
    nc.sync.dma_start(sin_sb[:], sinT.ap())
    nc.sync.dma_start(tri_sb[:], tri.ap())
    make_identity(nc, ident_f[:])
    nc.vector.tensor_copy(ident[:], ident_f[:])
    nc.vector.memset(ones_sb[64:65, :], 1.0)

    qT = bp.tile([128, S], BF16, tag="qT")
    kT = bp.tile([128, S], BF16, tag="kT")
    vt_sb = bp.tile([128, S], BF16, tag="vt_out")        # vT, later reused for outT
    v_all = bp.tile([128, NB128 * 130], BF16, tag="v_all")

    # ---------------- phase B: projections + interleaved RoPE ----------------
    # RoPE row layout: [h0_x1(32) | h0_x2(32) | h1_x1(32) | h1_x2(32)]; the
    # +- sign is folded into the host-built sin table (x1 rows carry -sin):
    #   ta = t * cos2;  tb = sw * sin2_signed;  t = ta + tb
    mul = mybir.AluOpType.mult
    HW = 2048  # RoPE free-dim chunk

    rope_pool = tc.tile_pool(name="rope", bufs=2)
    rp = rope_pool.__enter__()
    ctx_pools.append(rope_pool)

    def emit_rope(chunk):
        sl = slice(HW * chunk, HW * (chunk + 1))
        for ti, t in enumerate((qT, kT)):
            sw = rp.tile([128, HW], BF16, tag=("swq", "swk")[ti],
                         name=f"sw{ti}_{chunk}", bufs=2)
            for blk in range(4):
                dst = slice(32 * blk, 32 * blk + 32)
                srcs = slice(32 * (blk ^ 1), 32 * (blk ^ 1) + 32)
                nc.sync.dma_start(sw[dst, :], t[srcs, sl])
            ta = rp.tile([128, HW], BF16, tag="ta")
            tb = rp.tile([128, HW], BF16, tag="tb")
            nc.vector.tensor_tensor(ta[:], t[:, sl], cos_sb[:, sl], op=mul)
            nc.vector.tensor_tensor(tb[:], sw[:], sin_sb[:, sl], op=mul)
            nc.vector.tensor_tensor(t[:, sl], ta[:], tb[:],
                                    op=mybir.AluOpType.add)

    with tc.tile_pool(name="xp", bufs=12) as xp, \
         tc.tile_pool(name="prps", bufs=6, space="PSUM") as prps:
        for sb in range(NB512):
            xts = []
            for kc in range(8):
                xt = xp.tile([128, SQ], BF16, tag="x")
                nc.sync.dma_start(
                    xt[:], xT.ap()[128 * kc:128 * (kc + 1), SQ * sb:SQ * (sb + 1)])
                xts.append(xt)
            q_ps = prps.tile([128, SQ], F32, tag="pr")
            k_ps = prps.tile([128, SQ], F32, tag="pr")
            v_ps = prps.tile([128, SQ], F32, tag="pr")
            for kc in range(8):
                st, sp = (kc == 0), (kc == 7)
                nc.tensor.matmul(q_ps[:], wqk_sb[:, kc, 0:128], xts[kc][:],
                                 start=st, stop=sp)
                nc.tensor.matmul(k_ps[:], wqk_sb[:, kc, 128:256], xts[kc][:],
                                 start=st, stop=sp)
                nc.tensor.matmul(v_ps[:], wv_sb[:, kc, :], xts[kc][:],
                                 start=st, stop=sp)
            sl = slice(SQ * sb, SQ * (sb + 1))
            nc.scalar.copy(qT[:, sl], q_ps[:])
            nc.scalar.copy(kT[:, sl], k_ps[:])
            nc.vector.tensor_copy(vt_sb[:, sl], v_ps[:])
            if sb == 3:
                emit_rope(0)
        emit_rope(1)

    # ---------------- phase C: v^T -> v blocks (PE transpose) ----------------
    # ones columns for row-sums: v_all[:, 130j+64] = v_all[:, 130j+129] = 1.0
    onec = cp.tile([128, 1], BF16, tag="onec")
    nc.vector.memset(onec[:], 1.0)
    v_blk = v_all[:].rearrange("p (j c) -> p j c", c=130)
    nc.vector.tensor_copy(v_blk[:, :, 64], onec[:].broadcast_to([128, NB128]))
    nc.vector.tensor_copy(v_blk[:, :, 129], onec[:].broadcast_to([128, NB128]))
    with tc.tile_pool(name="vtp", bufs=4, space="PSUM") as vtp:
        for j in range(NB128):
            tp = vtp.tile([128, 128], BF16, tag="vt")
            nc.tensor.transpose(tp[:], vt_sb[:, 128 * j:128 * (j + 1)], ident[:])
            dst = v_all[:, 130 * j:130 * j + 130].rearrange(
                "p (t c) -> p t c", c=65)[:, :, 0:64]
            src = tp[:].rearrange("p (t c) -> p t c", c=64)
            nc.vector.tensor_copy(dst, src)

    if dbg is not None:
        nc.sync.dma_start(dbg["qT"].ap(), qT[:])
        nc.sync.dma_start(dbg["kT"].ap(), kT[:])
        nc.sync.dma_start(dbg["v_all"].ap(), v_all[:])

    # ---------------- phase D: attention + interleaved output projection ----
    outT = bp.tile([128, S], BF16, tag="vt_out")  # reuses vT slot
    exp = mybir.ActivationFunctionType.Exp
    with tc.tile_pool(name="scps", bufs=2, space="PSUM") as scps, \
         tc.tile_pool(name="smps", bufs=2, space="PSUM") as smps, \
         tc.tile_pool(name="yps", bufs=2, space="PSUM") as yps, \
         tc.tile_pool(name="ptp", bufs=3) as ptp, \
         tc.tile_pool(name="ysb", bufs=4) as ysb, \
         tc.tile_pool(name="recp", bufs=2) as recp:
        pending = [None]

        def emit_rec(out_ps_):
            """Start 1/rowsum as soon as the PV accumulation for this (b,h)
            has been emitted; runs on DVE + gpsimd, off the PE queue."""
            rec_f = recp.tile([65, SQ], F32, tag="rec_f", name="rec_f")
            rec = recp.tile([65, SQ], BF16, tag="rec", name="rec")
            nc.vector.reciprocal(rec_f[64:65, :], out_ps_[64:65, :])
            nc.vector.tensor_copy(rec[64:65, :], rec_f[64:65, :])
            return rec

        def flush_norm():
            if pending[0] is None:
                return
            h_, b_, out_ps_, rec = pending[0]
            pending[0] = None
            qsl_ = slice(SQ * b_, SQ * (b_ + 1))
            bc = yps.tile([64, SQ], F32, tag="y", name="bc")
            nc.tensor.matmul(bc[:], ones_sb[64:65, :], rec[64:65, :],
                             start=True, stop=True)
            if h_ == 0:
                dst = outT[0:64, qsl_]
                nc.vector.tensor_copy(dst, out_ps_[0:64, :])
                nc.vector.tensor_tensor(dst, dst, bc[:], op=mul)
            else:
                tmp64 = recp.tile([64, SQ], BF16, tag="tmp64", name="tmp64")
                nc.vector.tensor_copy(tmp64[:], out_ps_[0:64, :])
                nc.vector.tensor_tensor(tmp64[:], tmp64[:], bc[:], op=mul)
                nc.sync.dma_start(outT[64:128, qsl_], tmp64[:])
                # outT block b_ now complete -> emit its output projection
                for m in range(4 * b_, 4 * b_ + 4):
                    msl = slice(128 * m, 128 * (m + 1))
                    for nh in range(2):
                        nsl = slice(512 * nh, 512 * (nh + 1))
                        y_ps = yps.tile([128, SQ], F32, tag="y", name="y_ps")
                        nc.tensor.matmul(y_ps[:], outT[:, msl], wo_sb[:, nsl],
                                         start=True, stop=True)
                        y_sb = ysb.tile([128, SQ], F32, tag="ysb", name="y_sb")
                        nc.vector.tensor_copy(y_sb[:], y_ps[:])
                        nc.sync.dma_start(y.ap()[msl, nsl], y_sb[:])

        for b in (3, 7, 6, 5, 4, 2, 1, 0):
            nk = 4 * b + 4
            qsl = slice(SQ * b, SQ * (b + 1))
            for h in range(2):
                out_ps = smps.tile([65, SQ], F32, tag="sm")
                rh = slice(64 * h, 64 * h + 64)          # head h d_k rows
                for g0 in range(0, nk, GK):
                    gw = min(GK, nk - g0)
                    sc = scps.tile([128, GK * SQ], F32, tag="sc")
                    for j2 in range(gw):
                        k = g0 + j2
                        ksl = slice(SK * k, SK * (k + 1))
                        osl = slice(SQ * j2, SQ * (j2 + 1))
                        nc.tensor.matmul(sc[:, osl], kT[rh, ksl], qT[rh, qsl],
                                         start=True, stop=True)
                    pt = ptp.tile([128, GK * SQ], BF16, tag="pt")
                    nc.scalar.activation(pt[:, 0:SQ * gw], sc[:, 0:SQ * gw], exp,
                                         scale=0.125)
                    for j2 in range(gw):
                        k = g0 + j2
                        if k >= 4 * b:               # diagonal-straddling block
                            j = k - 4 * b
                            if j > 0:    # q < k for the first 128j columns
                                nc.gpsimd.memset(
                                    pt[:, SQ * j2:SQ * j2 + 128 * j], 0.0)
                            dsl = slice(SQ * j2 + 128 * j, SQ * j2 + 128 * j + 128)
                            nc.vector.tensor_tensor(pt[:, dsl], pt[:, dsl],
                                                    tri_sb[:], op=mul)
                    if g0 == GK:
                        flush_norm()
                    for j2 in range(gw):
                        k = g0 + j2
                        nc.tensor.matmul(out_ps[:],
                                         v_all[:, 130 * k + 65 * h:
                                               130 * k + 65 * h + 65],
                                         pt[:, SQ * j2:SQ * (j2 + 1)],
                                         start=(k == 0), stop=(k == nk - 1))
                rec = emit_rec(out_ps)
                pending[0] = (h, b, out_ps, rec)
        flush_norm()

    for p in reversed(ctx_pools):
        p.__exit__(None, None, None)


_CACHED = None


def _build():
    global _CACHED
    if _CACHED is not None:
        return _CACHED
    nc = bacc.Bacc("TRN2", target_bir_lowering=False, debug=False)
    xT = nc.dram_tensor("xT", [D, S], BF16, kind="ExternalInput")
    wqkT = nc.dram_tensor("wqkT", [D, 256], BF16, kind="ExternalInput")
    wvT = nc.dram_tensor("wvT", [D, 128], BF16, kind="ExternalInput")
    woT = nc.dram_tensor("woT", [128, D], BF16, kind="ExternalInput")
    cosT = nc.dram_tensor("cosT", [128, S], BF16, kind="ExternalInput")
    sinT = nc.dram_tensor("sinT", [128, S], BF16, kind="ExternalInput")
    tri = nc.dram_tensor("tri", [128, 128], BF16, kind="ExternalInput")
    y = nc.dram_tensor("y", [S, D], F32, kind="ExternalOutput")
    dbg = None
    if os.environ.get("KERN_DEBUG"):
        dbg = {
            "qT": nc.dram_tensor("dbg_qT", [128, S], BF16, kind="ExternalOutput"),
            "kT": nc.dram_tensor("dbg_kT", [128, S], BF16, kind="ExternalOutput"),
            "v_all": nc.dram_tensor("dbg_v_all", [128, NB128 * 130], BF16,
                                    kind="ExternalOutput"),
        }
    with tile.TileContext(nc) as tc:
        _emit(tc, xT, wqkT, wvT, woT, cosT, sinT, tri, y, dbg=dbg)
    nc.compile()
    _CACHED = nc
    return nc


def _host_prep(x, token_positions, Wq, Wk, Wv, Wo):
    x = np.asarray(x, dtype=np.float32).reshape(S, D)
    xT = np.ascontiguousarray(x.T).astype(NPBF16)

    pos = np.asarray(token_positions).reshape(S).astype(np.float32)
    inv = (np.float32(10000.0) **
           (-np.arange(0, DK // 2, dtype=np.float32) * np.float32(2.0 / DK)))
    ang = pos[None, :] * inv[:, None]          # [32, S]
    cosF = np.cos(ang).astype(np.float32)
    sinF = np.sin(ang).astype(np.float32)
    cosT = np.ascontiguousarray(np.tile(cosF, (4, 1))).astype(NPBF16)
    sinT = np.ascontiguousarray(np.tile(
        np.concatenate([-sinF, sinF], axis=0), (2, 1))).astype(NPBF16)  # signed

    ii = np.arange(128)[:, None]
    uu = np.arange(128)[None, :]
    tri = (uu >= ii).astype(NPBF16)             # [128, 128] triangle

    Wq = np.asarray(Wq, dtype=np.float32)
    Wk = np.asarray(Wk, dtype=np.float32)
    Wv = np.asarray(Wv, dtype=np.float32)
    Wo = np.asarray(Wo, dtype=np.float32)

    in_maps = []
    for c in range(N_CORES):
        idx = []
        for hl in range(2):   # per head: 32 even channels then 32 odd channels
            idx += [64 * (2 * c + hl) + 2 * j for j in range(32)]
            idx += [64 * (2 * c + hl) + 2 * j + 1 for j in range(32)]
        wq_c = Wq[idx, :]                       # [128, 1024]
        wk_c = Wk[idx, :]
        wqkT = np.ascontiguousarray(
            np.concatenate([wq_c.T, wk_c.T], axis=1)).astype(NPBF16)  # [1024, 256]
        wvT = np.ascontiguousarray(
            Wv[128 * c:128 * (c + 1), :].T).astype(NPBF16)  # [1024, 128]
        woT = np.ascontiguousarray(
            Wo[:, 128 * c:128 * (c + 1)].T).astype(NPBF16)  # [128, 1024]
        in_maps.append({
            "xT": xT, "wqkT": wqkT, "wvT": wvT, "woT": woT,
            "cosT": cosT, "sinT": sinT, "tri": tri,
        })
    return in_maps


def run(x, token_positions, Wq, Wk, Wv, Wo, trace=False):
    nc = _build()
    in_maps = _host_prep(x, token_positions, Wq, Wk, Wv, Wo)
    res = run_bass_kernel_spmd(nc, in_maps, core_ids=list(range(N_CORES)),
                               trace=trace)
    y = np.zeros((S, D), dtype=np.float32)
    for c in range(N_CORES):
        y += np.asarray(res.results[c]["y"], dtype=np.float32)
    return y.reshape(1, S, D), res


def kernel(x, token_positions, Wq, Wk, Wv, Wo):
    y, _ = run(x, token_positions, Wq, Wk, Wv, Wo)
    return y
